# revision 10
# baseline (speedup 1.0000x reference)
"""Trainium2 Bass kernel for EqualizedModConv2d (StyleGAN-style modulated conv).

Math (per sample b):
    s[b,i]   = sqrt(2) * leaky_relu(y[b] @ (mod_weight * dlat^-0.5).T + bias, 0.2)
    ww       = weight * s[b,:]                       (modulate, per in-channel)
    d[b,o]   = rsqrt(sum_{i,kh,kw} ww^2 + eps)       (demodulate, per out-channel)
    out[b]   = d[b,:] * conv2d(x[b], weight * s[b,:], SAME)

Factorization used on device (identical math):
    out[b,o,:] = d[b,o] * sum_{t,i} W_t[i,o] * (s[b,i] * xpad[b,i,:shifted(t)])
so the big conv runs with the *shared raw* weight as 9 accumulated matmul taps,
and the modulation/demodulation become tiny per-partition broadcast scalings of
the conv input/output. The O(B*C) style vectors s and d are computed on host
(0.01% of FLOPs); all heavy compute (38.7 GFLOP conv) runs on the NeuronCores.

Sharding: data-parallel over batch, 1 sample per core across 8 cores.
"""

import numpy as np
from contextlib import ExitStack

import ml_dtypes

import concourse.bass as bass
import concourse.mybir as mybir
import concourse.tile as tile
from concourse import bacc
from concourse.bass_utils import run_bass_kernel_spmd

# Problem shapes (hardcoded per contract).
B, C_IN, C_OUT, H, W, KS, DLAT = 8, 512, 512, 32, 32, 3, 512
HP, WP = H + 2, W + 2            # zero-padded input plane: 34 x 34
NPIX = HP * WP                   # 1156
PCH_I = C_IN // 128              # 4 input-channel chunks of 128 partitions
PCH_O = C_OUT // 128             # 4 output-channel chunks
NTAPS = KS * KS                  # 9 kernel taps
NW = NTAPS * PCH_I               # 36 lhsT tiles of [128, C_OUT]
NHALF = 2                        # output pixels split into 2 PSUM banks of 512
HHALF = H // NHALF               # 16 output rows per half
N_CORES = 8

SQRT2 = 1.4142135623730951
LRELU_SLOPE = 0.2
EPS = 1e-8

# Matmul precision: "f32r" (fp32 storage, reduced-precision PE pass, 1 cyc/row),
# "bf16" (cast inputs, 1 cyc/row), "f32" (exact, 4 cyc/row).
MM_DTYPE = "f32r"

_NC_CACHE: dict = {}


def _emit_conv(ctx, tc, o_d, w_d, x_d, s_d, dm_d, mm_dtype):
    nc = tc.nc
    f32 = mybir.dt.float32
    if mm_dtype == "f32r":
        xs_dt = w_sb_dt = mybir.dt.float32r
    elif mm_dtype == "bf16":
        xs_dt = w_sb_dt = mybir.dt.bfloat16
    else:
        xs_dt = w_sb_dt = f32

    singles = ctx.enter_context(tc.tile_pool(name="singles", bufs=1))
    psum = ctx.enter_context(tc.tile_pool(name="psum", bufs=8, space="PSUM"))
    outp = ctx.enter_context(tc.tile_pool(name="outp", bufs=4))

    # Per-partition modulation vectors: s (by in-channel), d (by out-channel).
    s_sb = singles.tile([128, PCH_I], f32)
    nc.sync.dma_start(s_sb[:, :], s_d[:, :])
    dm_sb = singles.tile([128, PCH_O], f32)
    nc.sync.dma_start(dm_sb[:, :], dm_d[:, :])

    # Input planes (pre-padded on host), then modulated copy xs = x * s.
    x_sb = singles.tile([128, PCH_I, HP, WP], f32)
    xs_sb = singles.tile([128, PCH_I, HP, WP], xs_dt)
    for c in range(PCH_I):
        nc.sync.dma_start(x_sb[:, c, :, :], x_d[c].rearrange("p (h w) -> p h w", h=HP))
        nc.vector.tensor_scalar_mul(
            xs_sb[:, c, :, :], x_sb[:, c, :, :], s_sb[:, c : c + 1]
        )

    # Conv weights in lhsT layout: tile ti = (tap, in-chunk) -> [128 (i), C_OUT (o)].
    w_sb = singles.tile([128, NW, C_OUT], w_sb_dt)
    for ti in range(NW):
        nc.sync.dma_start(w_sb[:, ti, :], w_d[ti])

    # 8 accumulation groups: (out-chunk mc, pixel-half nh), one PSUM bank each.
    ps = [[psum.tile([128, HHALF * W], f32, name="ps", tag="ps") for _ in range(NHALF)]
          for _ in range(PCH_O)]

    # Emission order: (tap, in-chunk) outer so PE consumes weight tiles in DMA
    # arrival order and all 8 PSUM groups accumulate in parallel.
    for t9 in range(NTAPS):
        kh, kw = divmod(t9, KS)
        for kc in range(PCH_I):
            ti = t9 * PCH_I + kc
            for mc in range(PCH_O):
                lhsT = w_sb[:, ti, mc * 128 : (mc + 1) * 128]
                for nh in range(NHALF):
                    r0 = nh * HHALF + kh
                    rhs = xs_sb[:, kc, r0 : r0 + HHALF, kw : kw + W]
                    nc.tensor.matmul(
                        ps[mc][nh][:, :],
                        lhsT=lhsT,
                        rhs=rhs,
                        start=(ti == 0),
                        stop=(ti == NW - 1),
                    )

    # Demodulate + store: out = psum * d[o].
    for mc in range(PCH_O):
        for nh in range(NHALF):
            ob = outp.tile([128, HHALF * W], f32, name="ob", tag="ob")
            nc.vector.tensor_scalar_mul(
                ob[:, :], ps[mc][nh][:, :], dm_sb[:, mc : mc + 1]
            )
            nc.sync.dma_start(
                o_d[mc * 128 : (mc + 1) * 128, nh * HHALF * W : (nh + 1) * HHALF * W],
                ob[:, :],
            )


def _build(mm_dtype):
    f32 = mybir.dt.float32
    if mm_dtype == "f32r":
        w_io_dt = mybir.dt.float32r
    elif mm_dtype == "bf16":
        w_io_dt = mybir.dt.bfloat16
    else:
        w_io_dt = f32
    nc = bacc.Bacc("TRN2", target_bir_lowering=False, debug=False)
    w_d = nc.dram_tensor("w", [NW, 128, C_OUT], w_io_dt, kind="ExternalInput").ap()
    x_d = nc.dram_tensor("x", [PCH_I, 128, NPIX], f32, kind="ExternalInput").ap()
    s_d = nc.dram_tensor("s", [128, PCH_I], f32, kind="ExternalInput").ap()
    dm_d = nc.dram_tensor("dm", [128, PCH_O], f32, kind="ExternalInput").ap()
    o_d = nc.dram_tensor("o", [C_OUT, H * W], f32, kind="ExternalOutput").ap()

    with tile.TileContext(nc) as tc:
        with ExitStack() as ctx:
            _emit_conv(ctx, tc, o_d, w_d, x_d, s_d, dm_d, mm_dtype)
    nc.compile()
    return nc


def get_nc(mm_dtype=MM_DTYPE):
    if mm_dtype not in _NC_CACHE:
        _NC_CACHE[mm_dtype] = _build(mm_dtype)
    return _NC_CACHE[mm_dtype]


def prepare_inputs(x, y, weight, mod_weight, bias, mm_dtype=MM_DTYPE):
    """Host-side prep: style/demod vectors + device data layouts."""
    x = np.asarray(x, np.float32)
    y = np.asarray(y, np.float32)
    weight = np.asarray(weight, np.float32)
    mod_weight = np.asarray(mod_weight, np.float32)
    bias = np.asarray(bias, np.float32)

    # Style s[b,i] = sqrt(2) * leaky_relu(y @ (mod_weight * dlat^-0.5).T + bias).
    s = y @ (mod_weight.T * np.float32(DLAT ** -0.5))
    s = s + bias[None, :]
    s = np.where(s >= 0, s, LRELU_SLOPE * s).astype(np.float32) * np.float32(SQRT2)

    # Demod d[b,o] = rsqrt(sum_i s^2 * sum_t w^2 + eps) (exact refactoring).
    w64 = weight.astype(np.float64)
    w2 = (w64 * w64).sum(axis=(2, 3))                       # [C_OUT, C_IN]
    den = (s.astype(np.float64) ** 2) @ w2.T + EPS          # [B, C_OUT]
    dmod = (1.0 / np.sqrt(den)).astype(np.float32)

    # lhsT weights: [kh, kw, i, o] -> [36, 128, C_OUT].
    wT = np.ascontiguousarray(weight.transpose(2, 3, 1, 0)).reshape(NW, 128, C_OUT)
    if mm_dtype == "bf16":
        wT = wT.astype(ml_dtypes.bfloat16)

    # Zero-padded input planes, chunked: [B, 4, 128, 1156].
    xp = np.zeros((B, C_IN, HP, WP), np.float32)
    xp[:, :, 1 : H + 1, 1 : W + 1] = x
    xp = xp.reshape(B, PCH_I, 128, NPIX)

    # Per-partition layouts for the broadcast scalings: [B, 128, 4].
    s_t = np.ascontiguousarray(s.reshape(B, PCH_I, 128).transpose(0, 2, 1))
    d_t = np.ascontiguousarray(dmod.reshape(B, PCH_O, 128).transpose(0, 2, 1))

    return [
        {"w": wT, "x": xp[b], "s": s_t[b], "dm": d_t[b]}
        for b in range(B)
    ]


def kernel(x, y, weight, mod_weight, bias):
    in_maps = prepare_inputs(x, y, weight, mod_weight, bias, MM_DTYPE)
    nc = get_nc(MM_DTYPE)
    res = run_bass_kernel_spmd(nc, in_maps, core_ids=list(range(N_CORES)))
    return np.stack(
        [res.results[b]["o"].reshape(C_OUT, H, W) for b in range(B)]
    ).astype(np.float32)


# revision 14
# speedup vs baseline: 3.2605x; 3.2605x over previous
"""Trainium2 Bass kernel for EqualizedModConv2d (StyleGAN-style modulated conv).

Math (per sample b):
    s[b,i]   = sqrt(2) * leaky_relu(y[b] @ (mod_weight * dlat^-0.5).T + bias, 0.2)
    ww       = weight * s[b,:]                       (modulate, per in-channel)
    d[b,o]   = rsqrt(sum_{i,kh,kw} ww^2 + eps)       (demodulate, per out-channel)
    out[b]   = d[b,:] * conv2d(x[b], weight * s[b,:], SAME)

Factorization used on device (identical math):
    out[b,o,:] = d[b,o] * sum_{t,i} W_t[i,o] * (s[b,i] * xpad[b,i,:shifted(t)])
so the big conv runs with the *shared raw* weight as 9 accumulated matmul taps,
and the modulation/demodulation become tiny per-partition broadcast scalings of
the conv input/output. The O(B*C) style vectors s and d are computed on host
(0.01% of FLOPs); all heavy compute (38.7 GFLOP conv) runs on the NeuronCores.

Sharding: data-parallel over batch, 1 sample per NeuronCore across 8 cores.
The conv weight is broadcast (replicated) to all cores; per-core inputs are the
sample's image plus its style/demod vectors.
"""

import hashlib
import numpy as np
from contextlib import ExitStack

import concourse.bass as bass
import concourse.mybir as mybir
import concourse.tile as tile
from concourse import bacc

# Problem shapes (hardcoded per contract).
B, C_IN, C_OUT, H, W, KS, DLAT = 8, 512, 512, 32, 32, 3, 512
HP, WP = H + 2, W + 2            # zero-padded input plane: 34 x 34
NPIX = H * W                     # 1024 (unpadded, as transferred)
PCH_I = C_IN // 128              # 4 input-channel chunks of 128 partitions
PCH_O = C_OUT // 128             # 4 output-channel chunks
NTAPS = KS * KS                  # 9 kernel taps
NW = NTAPS * PCH_I               # 36 lhsT tiles of [128, C_OUT]
NHALF = 2                        # output pixels split into 2 PSUM banks of 512
HHALF = H // NHALF               # 16 output rows per half
N_CORES = 8

SQRT2 = 1.4142135623730951
LRELU_SLOPE = 0.2
EPS = 1e-8

# Matmul precision: "f32r" (fp32 storage, reduced-precision PE pass, 1 cyc/row),
# "bf16" (cast inputs, 1 cyc/row), "f32" (exact, 4 cyc/row).
MM_DTYPE = "f32r"

_NC_CACHE: dict = {}
_RUNNER_CACHE: dict = {}
_W_DEV_CACHE: dict = {}


def _mm_dts(mm_dtype):
    if mm_dtype == "f32r":
        return mybir.dt.float32r
    if mm_dtype == "bf16":
        return mybir.dt.bfloat16
    return mybir.dt.float32


def _emit_conv(ctx, tc, o_d, w_d, x_d, s_d, dm_d, mm_dtype):
    nc = tc.nc
    f32 = mybir.dt.float32
    mm_dt = _mm_dts(mm_dtype)

    singles = ctx.enter_context(tc.tile_pool(name="singles", bufs=1))
    psum = ctx.enter_context(tc.tile_pool(name="psum", bufs=8, space="PSUM"))
    outp = ctx.enter_context(tc.tile_pool(name="outp", bufs=4))

    # Per-partition modulation vectors: s (by in-channel), d (by out-channel).
    s_sb = singles.tile([128, PCH_I], f32)
    nc.sync.dma_start(s_sb[:, :], s_d[:, :])
    dm_sb = singles.tile([128, PCH_O], f32)
    nc.sync.dma_start(dm_sb[:, :], dm_d[:, :])

    # Input planes: DMA unpadded, zero-pad + modulate into xs = s * x.
    x_sb = singles.tile([128, PCH_I, H, W], f32)
    xs_sb = singles.tile([128, PCH_I, HP, WP], mm_dt)
    xs_memset_view = (
        xs_sb.bitcast(f32) if mm_dtype == "f32r" else xs_sb
    )
    for c in range(PCH_I):
        nc.vector.memset(xs_memset_view[:, c, :, :], 0.0)
        nc.sync.dma_start(x_sb[:, c, :, :], x_d[c].rearrange("p (h w) -> p h w", h=H))
        nc.vector.tensor_scalar_mul(
            xs_sb[:, c, 1 : H + 1, 1 : W + 1], x_sb[:, c, :, :], s_sb[:, c : c + 1]
        )

    # Conv weights in lhsT layout: tile ti = (tap, in-chunk) -> [128 (i), C_OUT (o)].
    w_sb = singles.tile([128, NW, C_OUT], mm_dt)
    for ti in range(NW):
        nc.sync.dma_start(w_sb[:, ti, :], w_d[ti])

    # 8 accumulation groups: (out-chunk mc, pixel-half nh), one PSUM bank each.
    ps = [[psum.tile([128, HHALF * W], f32, name="ps", tag="ps") for _ in range(NHALF)]
          for _ in range(PCH_O)]

    # Emission order: (tap, in-chunk) outer so PE consumes weight tiles in DMA
    # arrival order and all 8 PSUM groups accumulate in parallel.
    for t9 in range(NTAPS):
        kh, kw = divmod(t9, KS)
        for kc in range(PCH_I):
            ti = t9 * PCH_I + kc
            for mc in range(PCH_O):
                lhsT = w_sb[:, ti, mc * 128 : (mc + 1) * 128]
                for nh in range(NHALF):
                    r0 = nh * HHALF + kh
                    rhs = xs_sb[:, kc, r0 : r0 + HHALF, kw : kw + W]
                    nc.tensor.matmul(
                        ps[mc][nh][:, :],
                        lhsT=lhsT,
                        rhs=rhs,
                        start=(ti == 0),
                        stop=(ti == NW - 1),
                    )

    # Demodulate + store: out = psum * d[o].
    for mc in range(PCH_O):
        for nh in range(NHALF):
            ob = outp.tile([128, HHALF * W], f32, name="ob", tag="ob")
            nc.vector.tensor_scalar_mul(
                ob[:, :], ps[mc][nh][:, :], dm_sb[:, mc : mc + 1]
            )
            nc.sync.dma_start(
                o_d[mc * 128 : (mc + 1) * 128, nh * HHALF * W : (nh + 1) * HHALF * W],
                ob[:, :],
            )


def _build(mm_dtype):
    f32 = mybir.dt.float32
    w_io_dt = _mm_dts(mm_dtype) if mm_dtype != "f32" else f32
    nc = bacc.Bacc("TRN2", target_bir_lowering=False, debug=False)
    w_d = nc.dram_tensor("w", [NW, 128, C_OUT], w_io_dt, kind="ExternalInput").ap()
    x_d = nc.dram_tensor("x", [PCH_I, 128, NPIX], f32, kind="ExternalInput").ap()
    s_d = nc.dram_tensor("s", [128, PCH_I], f32, kind="ExternalInput").ap()
    dm_d = nc.dram_tensor("dm", [128, PCH_O], f32, kind="ExternalInput").ap()
    o_d = nc.dram_tensor("o", [C_OUT, H * W], f32, kind="ExternalOutput").ap()

    with tile.TileContext(nc) as tc:
        with ExitStack() as ctx:
            _emit_conv(ctx, tc, o_d, w_d, x_d, s_d, dm_d, mm_dtype)
    nc.compile()
    return nc


def get_nc(mm_dtype=MM_DTYPE):
    if mm_dtype not in _NC_CACHE:
        _NC_CACHE[mm_dtype] = _build(mm_dtype)
    return _NC_CACHE[mm_dtype]


def prepare_inputs(x, y, weight, mod_weight, bias, mm_dtype=MM_DTYPE):
    """Host-side prep: style/demod vectors + device data layouts.

    Returns a dict of *global* arrays: "w" replicated, others concatenated
    along axis 0 across the 8 cores (one sample per core).
    """
    x = np.ascontiguousarray(np.asarray(x, np.float32))
    y = np.asarray(y, np.float32)
    weight = np.asarray(weight, np.float32)
    mod_weight = np.asarray(mod_weight, np.float32)
    bias = np.asarray(bias, np.float32)

    # Style s[b,i] = sqrt(2) * leaky_relu(y @ (mod_weight * dlat^-0.5).T + bias).
    s = y @ (mod_weight.T * np.float32(DLAT ** -0.5))
    s = s + bias[None, :]
    s = np.where(s >= 0, s, LRELU_SLOPE * s).astype(np.float32) * np.float32(SQRT2)

    # Demod d[b,o] = rsqrt(sum_i s^2 * sum_t w^2 + eps) (exact refactoring).
    w64 = weight.astype(np.float64)
    w2 = (w64 * w64).sum(axis=(2, 3))                       # [C_OUT, C_IN]
    den = (s.astype(np.float64) ** 2) @ w2.T + EPS          # [B, C_OUT]
    dmod = (1.0 / np.sqrt(den)).astype(np.float32)

    # lhsT weights: [kh, kw, i, o] -> [36, 128, C_OUT]; shared by all cores.
    wT = np.ascontiguousarray(weight.transpose(2, 3, 1, 0)).reshape(NW, 128, C_OUT)
    if mm_dtype == "bf16":
        import ml_dtypes

        wT = wT.astype(ml_dtypes.bfloat16)

    xg = x.reshape(B * PCH_I, 128, NPIX)                    # [32, 128, 1024]
    s_t = np.ascontiguousarray(
        s.reshape(B, PCH_I, 128).transpose(0, 2, 1)
    ).reshape(B * 128, PCH_I)
    d_t = np.ascontiguousarray(
        dmod.reshape(B, PCH_O, 128).transpose(0, 2, 1)
    ).reshape(B * 128, PCH_O)

    return {"w": wT, "x": xg, "s": s_t, "dm": d_t}


def per_core_map(global_in, core):
    """Slice the global input dict into one core's input map (for CoreSim)."""
    return {
        "w": global_in["w"],
        "x": global_in["x"][core * PCH_I : (core + 1) * PCH_I],
        "s": global_in["s"][core * 128 : (core + 1) * 128],
        "dm": global_in["dm"][core * 128 : (core + 1) * 128],
    }


def _make_runner(nc):
    """Persistent jitted SPMD executor: weight replicated, rest batch-sharded,
    donated output zeros created on-device (nothing extra over the wire)."""
    import jax
    import jax.numpy as jnp
    from jax.sharding import Mesh, PartitionSpec
    from jax.experimental.shard_map import shard_map

    from concourse.bass2jax import (
        _bass_exec_p,
        install_neuronx_cc_hook,
        partition_id_tensor,
    )

    install_neuronx_cc_hook()

    partition_name = (
        nc.partition_id_tensor.name if nc.partition_id_tensor is not None else None
    )
    in_names: list = []
    out_names: list = []
    out_avals: list = []
    for alloc in nc.m.functions[0].allocations:
        if not isinstance(alloc, mybir.MemoryLocationSet):
            continue
        name = alloc.memorylocations[0].name
        if alloc.kind == "ExternalInput":
            if name != partition_name:
                in_names.append(name)
        elif alloc.kind == "ExternalOutput":
            out_names.append(name)
            out_avals.append(
                jax.core.ShapedArray(tuple(alloc.tensor_shape), mybir.dt.np(alloc.dtype))
            )
    all_in_names = list(in_names) + list(out_names)
    if partition_name is not None:
        all_in_names.append(partition_name)

    def _body(*args):
        operands = list(args)
        if partition_name is not None:
            operands.append(partition_id_tensor())
        outs = _bass_exec_p.bind(
            *operands,
            out_avals=tuple(out_avals),
            in_names=tuple(all_in_names),
            out_names=tuple(out_names),
            lowering_input_output_aliases=(),
            sim_require_finite=True,
            sim_require_nnan=True,
            nc=nc,
        )
        return tuple(outs)

    devices = jax.devices()[:N_CORES]
    assert len(devices) == N_CORES, f"need {N_CORES} devices, got {len(devices)}"
    mesh = Mesh(np.asarray(devices), ("core",))
    spec_by_name = {n: PartitionSpec("core") for n in in_names}
    spec_by_name["w"] = PartitionSpec()          # replicated: one copy over the wire
    in_specs = tuple(spec_by_name[n] for n in in_names) + (
        (PartitionSpec("core"),) * len(out_names)
    )
    out_specs = (PartitionSpec("core"),) * len(out_names)
    fn = jax.jit(
        shard_map(
            _body, mesh=mesh, in_specs=in_specs, out_specs=out_specs, check_rep=False
        )
    )

    # Output "seed" buffers: the bass_exec custom call requires one parameter
    # per ExternalOutput. Our kernel writes every output element, so they only
    # need to exist, not be re-zeroed per call — create once on-device.
    from jax.sharding import NamedSharding

    def _mk_zeros():
        return tuple(
            jnp.zeros((N_CORES * a.shape[0],) + a.shape[1:], a.dtype)
            for a in out_avals
        )

    zeros_sh = tuple(
        NamedSharding(mesh, PartitionSpec("core")) for _ in out_avals
    )
    out_seeds = jax.jit(_mk_zeros, out_shardings=zeros_sh)()
    return fn, in_names, out_names, mesh, out_seeds


def get_runner(mm_dtype=MM_DTYPE):
    if mm_dtype not in _RUNNER_CACHE:
        _RUNNER_CACHE[mm_dtype] = _make_runner(get_nc(mm_dtype))
    return _RUNNER_CACHE[mm_dtype]


def _w_device(wT, mesh):
    """Cache the replicated weight on-device across calls (keyed by content)."""
    import jax
    from jax.sharding import NamedSharding, PartitionSpec

    key = hashlib.blake2b(wT.tobytes(), digest_size=16).hexdigest()
    hit = _W_DEV_CACHE.get(key)
    if hit is None:
        sh = NamedSharding(mesh, PartitionSpec())
        _W_DEV_CACHE.clear()
        hit = _W_DEV_CACHE[key] = jax.device_put(wT, sh)
    return hit


def kernel(x, y, weight, mod_weight, bias):
    gin = prepare_inputs(x, y, weight, mod_weight, bias, MM_DTYPE)
    fn, in_names, out_names, mesh, out_seeds = get_runner(MM_DTYPE)
    gin["w"] = _w_device(gin["w"], mesh)
    outs = fn(*[gin[n] for n in in_names], *out_seeds)
    out = np.asarray(outs[out_names.index("o")])             # [8*512, 1024]
    return out.reshape(B, C_OUT, H, W).astype(np.float32, copy=False)


# revision 17
# speedup vs baseline: 3.2999x; 1.0121x over previous
"""Trainium2 Bass kernel for EqualizedModConv2d (StyleGAN-style modulated conv).

Math (per sample b):
    s[b,i]   = sqrt(2) * leaky_relu(y[b] @ (mod_weight * dlat^-0.5).T + bias, 0.2)
    ww       = weight * s[b,:]                       (modulate, per in-channel)
    d[b,o]   = rsqrt(sum_{i,kh,kw} ww^2 + eps)       (demodulate, per out-channel)
    out[b]   = d[b,:] * conv2d(x[b], weight * s[b,:], SAME)

Factorization used on device (identical math):
    out[b,o,:] = d[b,o] * sum_{t,i} W_t[i,o] * (s[b,i] * xpad[b,i,:shifted(t)])
so the big conv runs with the *shared raw* weight as 9 accumulated matmul taps,
and the modulation/demodulation become tiny per-partition broadcast scalings of
the conv input/output. The O(B*C) style vectors s and d are computed on host
(0.01% of FLOPs); all heavy compute (38.7 GFLOP conv) runs on the NeuronCores.

Sharding: data-parallel over batch, 1 sample per NeuronCore across 8 cores.
The conv weight is broadcast (replicated) to all cores; per-core inputs are the
sample's image plus its style/demod vectors.
"""

import hashlib
import numpy as np
from contextlib import ExitStack

import concourse.bass as bass
import concourse.mybir as mybir
import concourse.tile as tile
from concourse import bacc

# Problem shapes (hardcoded per contract).
B, C_IN, C_OUT, H, W, KS, DLAT = 8, 512, 512, 32, 32, 3, 512
HP, WP = H + 2, W + 2            # zero-padded input plane: 34 x 34
NPIX = H * W                     # 1024 (unpadded, as transferred)
PCH_I = C_IN // 128              # 4 input-channel chunks of 128 partitions
PCH_O = C_OUT // 128             # 4 output-channel chunks
NTAPS = KS * KS                  # 9 kernel taps
NW = NTAPS * PCH_I               # 36 lhsT tiles of [128, C_OUT]
NHALF = 2                        # output pixels split into 2 PSUM banks of 512
HHALF = H // NHALF               # 16 output rows per half
N_CORES = 8

SQRT2 = 1.4142135623730951
LRELU_SLOPE = 0.2
EPS = 1e-8

# Matmul precision: "f32r" (fp32 storage, reduced-precision PE pass, 1 cyc/row),
# "bf16" (cast inputs, 1 cyc/row), "f32" (exact, 4 cyc/row).
MM_DTYPE = "f32r"

_NC_CACHE: dict = {}
_RUNNER_CACHE: dict = {}
_W_DEV_CACHE: dict = {}


def _mm_dts(mm_dtype):
    if mm_dtype == "f32r":
        return mybir.dt.float32r
    if mm_dtype == "bf16":
        return mybir.dt.bfloat16
    return mybir.dt.float32


def _emit_conv(ctx, tc, o_d, w_d, x_d, sdm_d, mm_dtype):
    nc = tc.nc
    f32 = mybir.dt.float32
    mm_dt = _mm_dts(mm_dtype)

    singles = ctx.enter_context(tc.tile_pool(name="singles", bufs=1))
    psum = ctx.enter_context(tc.tile_pool(name="psum", bufs=8, space="PSUM"))
    outp = ctx.enter_context(tc.tile_pool(name="outp", bufs=4))

    # Per-partition modulation vectors in one tiny DMA: cols [0:4]=s, [4:8]=d.
    sdm_sb = singles.tile([128, PCH_I + PCH_O], f32)
    nc.sync.dma_start(sdm_sb[:, :], sdm_d[:, :])

    # Input planes: DMA unpadded; zero only the pad border (GpSimd — otherwise
    # idle), then modulate the interior: xs = s * x.
    x_sb = singles.tile([128, PCH_I, H, W], f32)
    xs_sb = singles.tile([128, PCH_I, HP, WP], mm_dt)
    xs_z = xs_sb.bitcast(f32) if mm_dtype == "f32r" else xs_sb
    for c in range(PCH_I):
        nc.gpsimd.memset(xs_z[:, c, 0, :], 0.0)
        nc.gpsimd.memset(xs_z[:, c, HP - 1, :], 0.0)
        nc.gpsimd.memset(xs_z[:, c, 1 : HP - 1, 0:1], 0.0)
        nc.gpsimd.memset(xs_z[:, c, 1 : HP - 1, WP - 1 : WP], 0.0)

    # Conv weights in lhsT layout, in-chunk-major: ti = kc*9 + tap.
    # First chunk arrives as 9 small per-tap DMAs (PE can start ~immediately);
    # remaining chunks as 3 large block DMAs (cheap to issue, arrive in time).
    w_sb = singles.tile([128, NW, C_OUT], mm_dt)

    def x_load(c):
        nc.sync.dma_start(x_sb[:, c, :, :], x_d[c].rearrange("p (h w) -> p h w", h=H))
        nc.vector.tensor_scalar_mul(
            xs_sb[:, c, 1 : H + 1, 1 : W + 1], x_sb[:, c, :, :], sdm_sb[:, c : c + 1]
        )

    # Interleave so the critical chain (x0, w[0,0]) issues first.
    x_load(0)
    nc.sync.dma_start(w_sb[:, 0, :], w_d[0, 0])
    x_load(1)
    nc.sync.dma_start(w_sb[:, 1, :], w_d[0, 1])
    x_load(2)
    nc.sync.dma_start(w_sb[:, 2, :], w_d[0, 2])
    x_load(3)
    for t9 in range(3, NTAPS):
        nc.sync.dma_start(w_sb[:, t9, :], w_d[0, t9])
    for kc in range(1, PCH_I):
        nc.sync.dma_start(
            w_sb[:, kc * NTAPS : (kc + 1) * NTAPS, :],
            w_d[kc].rearrange("t p n -> p t n"),
        )

    # 8 accumulation groups: (out-chunk mc, pixel-half nh), one PSUM bank each.
    ps = [[psum.tile([128, HHALF * W], f32, name="ps", tag="ps") for _ in range(NHALF)]
          for _ in range(PCH_O)]

    def mm(mc, nh, kc, t9, start, stop):
        kh, kw = divmod(t9, KS)
        r0 = nh * HHALF + kh
        nc.tensor.matmul(
            ps[mc][nh][:, :],
            lhsT=w_sb[:, kc * NTAPS + t9, mc * 128 : (mc + 1) * 128],
            rhs=xs_sb[:, kc, r0 : r0 + HHALF, kw : kw + W],
            start=start,
            stop=stop,
        )

    # Phase A: chunks 0..2 in weight-arrival order, all 8 groups in parallel
    # (PE stays dense and consumes w tiles as the DMAs land).
    for kc in range(PCH_I - 1):
        for t9 in range(NTAPS):
            for mc in range(PCH_O):
                for nh in range(NHALF):
                    mm(mc, nh, kc, t9, start=(kc == 0 and t9 == 0), stop=False)

    # Phase B: last chunk group-by-group so groups finish staggered ~1.9us
    # apart and the demod+store drains overlap with remaining PE work.
    kc = PCH_I - 1
    for mc in range(PCH_O):
        for nh in range(NHALF):
            for t9 in range(NTAPS):
                mm(mc, nh, kc, t9, start=False, stop=(t9 == NTAPS - 1))
            ob = outp.tile([128, HHALF * W], f32, name="ob", tag="ob")
            nc.vector.tensor_scalar_mul(
                ob[:, :], ps[mc][nh][:, :], sdm_sb[:, PCH_I + mc : PCH_I + mc + 1]
            )
            nc.sync.dma_start(
                o_d[mc * 128 : (mc + 1) * 128, nh * HHALF * W : (nh + 1) * HHALF * W],
                ob[:, :],
            )


def _build(mm_dtype):
    f32 = mybir.dt.float32
    w_io_dt = _mm_dts(mm_dtype) if mm_dtype != "f32" else f32
    nc = bacc.Bacc("TRN2", target_bir_lowering=False, debug=False)
    w_d = nc.dram_tensor(
        "w", [PCH_I, NTAPS, 128, C_OUT], w_io_dt, kind="ExternalInput"
    ).ap()
    x_d = nc.dram_tensor("x", [PCH_I, 128, NPIX], f32, kind="ExternalInput").ap()
    sdm_d = nc.dram_tensor(
        "sdm", [128, PCH_I + PCH_O], f32, kind="ExternalInput"
    ).ap()
    o_d = nc.dram_tensor("o", [C_OUT, H * W], f32, kind="ExternalOutput").ap()

    with tile.TileContext(nc) as tc:
        with ExitStack() as ctx:
            _emit_conv(ctx, tc, o_d, w_d, x_d, sdm_d, mm_dtype)
    nc.compile()
    return nc


def get_nc(mm_dtype=MM_DTYPE):
    if mm_dtype not in _NC_CACHE:
        _NC_CACHE[mm_dtype] = _build(mm_dtype)
    return _NC_CACHE[mm_dtype]


def prepare_inputs(x, y, weight, mod_weight, bias, mm_dtype=MM_DTYPE):
    """Host-side prep: style/demod vectors + device data layouts.

    Returns a dict of *global* arrays: "w" replicated, others concatenated
    along axis 0 across the 8 cores (one sample per core).
    """
    x = np.ascontiguousarray(np.asarray(x, np.float32))
    y = np.asarray(y, np.float32)
    weight = np.asarray(weight, np.float32)
    mod_weight = np.asarray(mod_weight, np.float32)
    bias = np.asarray(bias, np.float32)

    # Style s[b,i] = sqrt(2) * leaky_relu(y @ (mod_weight * dlat^-0.5).T + bias).
    s = y @ (mod_weight.T * np.float32(DLAT ** -0.5))
    s = s + bias[None, :]
    s = np.where(s >= 0, s, LRELU_SLOPE * s).astype(np.float32) * np.float32(SQRT2)

    # Demod d[b,o] = rsqrt(sum_i s^2 * sum_t w^2 + eps) (exact refactoring).
    w64 = weight.astype(np.float64)
    w2 = (w64 * w64).sum(axis=(2, 3))                       # [C_OUT, C_IN]
    den = (s.astype(np.float64) ** 2) @ w2.T + EPS          # [B, C_OUT]
    dmod = (1.0 / np.sqrt(den)).astype(np.float32)

    # lhsT weights [kh,kw,i,o], in-chunk-major: [PCH_I, NTAPS, 128, C_OUT].
    wT = np.ascontiguousarray(
        weight.transpose(2, 3, 1, 0)
        .reshape(NTAPS, PCH_I, 128, C_OUT)
        .transpose(1, 0, 2, 3)
    )
    if mm_dtype == "bf16":
        import ml_dtypes

        wT = wT.astype(ml_dtypes.bfloat16)

    xg = x.reshape(B * PCH_I, 128, NPIX)                    # [32, 128, 1024]
    s_t = s.reshape(B, PCH_I, 128).transpose(0, 2, 1)       # [B, 128, 4]
    d_t = dmod.reshape(B, PCH_O, 128).transpose(0, 2, 1)    # [B, 128, 4]
    sdm = np.ascontiguousarray(np.concatenate([s_t, d_t], axis=2)).reshape(
        B * 128, PCH_I + PCH_O
    )

    return {"w": wT, "x": xg, "sdm": sdm}


def per_core_map(global_in, core):
    """Slice the global input dict into one core's input map (for CoreSim)."""
    return {
        "w": global_in["w"],
        "x": global_in["x"][core * PCH_I : (core + 1) * PCH_I],
        "sdm": global_in["sdm"][core * 128 : (core + 1) * 128],
    }


def _make_runner(nc):
    """Persistent jitted SPMD executor: weight replicated, rest batch-sharded,
    donated output zeros created on-device (nothing extra over the wire)."""
    import jax
    import jax.numpy as jnp
    from jax.sharding import Mesh, PartitionSpec
    from jax.experimental.shard_map import shard_map

    from concourse.bass2jax import (
        _bass_exec_p,
        install_neuronx_cc_hook,
        partition_id_tensor,
    )

    install_neuronx_cc_hook()

    partition_name = (
        nc.partition_id_tensor.name if nc.partition_id_tensor is not None else None
    )
    in_names: list = []
    out_names: list = []
    out_avals: list = []
    for alloc in nc.m.functions[0].allocations:
        if not isinstance(alloc, mybir.MemoryLocationSet):
            continue
        name = alloc.memorylocations[0].name
        if alloc.kind == "ExternalInput":
            if name != partition_name:
                in_names.append(name)
        elif alloc.kind == "ExternalOutput":
            out_names.append(name)
            out_avals.append(
                jax.core.ShapedArray(tuple(alloc.tensor_shape), mybir.dt.np(alloc.dtype))
            )
    all_in_names = list(in_names) + list(out_names)
    if partition_name is not None:
        all_in_names.append(partition_name)

    def _body(*args):
        operands = list(args)
        if partition_name is not None:
            operands.append(partition_id_tensor())
        outs = _bass_exec_p.bind(
            *operands,
            out_avals=tuple(out_avals),
            in_names=tuple(all_in_names),
            out_names=tuple(out_names),
            lowering_input_output_aliases=(),
            sim_require_finite=True,
            sim_require_nnan=True,
            nc=nc,
        )
        return tuple(outs)

    devices = jax.devices()[:N_CORES]
    assert len(devices) == N_CORES, f"need {N_CORES} devices, got {len(devices)}"
    mesh = Mesh(np.asarray(devices), ("core",))
    spec_by_name = {n: PartitionSpec("core") for n in in_names}
    spec_by_name["w"] = PartitionSpec()          # replicated: one copy over the wire
    in_specs = tuple(spec_by_name[n] for n in in_names) + (
        (PartitionSpec("core"),) * len(out_names)
    )
    out_specs = (PartitionSpec("core"),) * len(out_names)
    fn = jax.jit(
        shard_map(
            _body, mesh=mesh, in_specs=in_specs, out_specs=out_specs, check_rep=False
        )
    )

    # Output "seed" buffers: the bass_exec custom call requires one parameter
    # per ExternalOutput. Our kernel writes every output element, so they only
    # need to exist, not be re-zeroed per call — create once on-device.
    from jax.sharding import NamedSharding

    def _mk_zeros():
        return tuple(
            jnp.zeros((N_CORES * a.shape[0],) + a.shape[1:], a.dtype)
            for a in out_avals
        )

    zeros_sh = tuple(
        NamedSharding(mesh, PartitionSpec("core")) for _ in out_avals
    )
    out_seeds = jax.jit(_mk_zeros, out_shardings=zeros_sh)()
    return fn, in_names, out_names, mesh, out_seeds


def get_runner(mm_dtype=MM_DTYPE):
    if mm_dtype not in _RUNNER_CACHE:
        _RUNNER_CACHE[mm_dtype] = _make_runner(get_nc(mm_dtype))
    return _RUNNER_CACHE[mm_dtype]


def _w_device(wT, mesh):
    """Cache the replicated weight on-device across calls (keyed by content)."""
    import jax
    from jax.sharding import NamedSharding, PartitionSpec

    key = hashlib.blake2b(wT.tobytes(), digest_size=16).hexdigest()
    hit = _W_DEV_CACHE.get(key)
    if hit is None:
        sh = NamedSharding(mesh, PartitionSpec())
        _W_DEV_CACHE.clear()
        hit = _W_DEV_CACHE[key] = jax.device_put(wT, sh)
    return hit


def kernel(x, y, weight, mod_weight, bias):
    gin = prepare_inputs(x, y, weight, mod_weight, bias, MM_DTYPE)
    fn, in_names, out_names, mesh, out_seeds = get_runner(MM_DTYPE)
    gin["w"] = _w_device(gin["w"], mesh)
    outs = fn(*[gin[n] for n in in_names], *out_seeds)
    out = np.asarray(outs[out_names.index("o")])             # [8*512, 1024]
    return out.reshape(B, C_OUT, H, W).astype(np.float32, copy=False)


# revision 19
# speedup vs baseline: 41.2096x; 12.4880x over previous
"""Trainium2 Bass kernel for EqualizedModConv2d (StyleGAN-style modulated conv).

Math (per sample b):
    s[b,i]   = sqrt(2) * leaky_relu(y[b] @ (mod_weight * dlat^-0.5).T + bias, 0.2)
    ww       = weight * s[b,:]                       (modulate, per in-channel)
    d[b,o]   = rsqrt(sum_{i,kh,kw} ww^2 + eps)       (demodulate, per out-channel)
    out[b]   = d[b,:] * conv2d(x[b], weight * s[b,:], SAME)

Factorization used on device (identical math):
    out[b,o,:] = d[b,o] * sum_{t,i} W_t[i,o] * (s[b,i] * xpad[b,i,:shifted(t)])
so the big conv runs with the *shared raw* weight as 9 accumulated matmul taps,
and the modulation/demodulation become tiny per-partition broadcast scalings of
the conv input/output. The O(B*C) style vectors s and d are computed on host
(0.01% of FLOPs); all heavy compute (38.7 GFLOP conv) runs on the NeuronCores.

Sharding: data-parallel over batch, 1 sample per NeuronCore across 8 cores.
The conv weight is broadcast (replicated) to all cores; per-core inputs are the
sample's image plus its style/demod vectors.
"""

import hashlib
import numpy as np
from contextlib import ExitStack

import concourse.bass as bass
import concourse.mybir as mybir
import concourse.tile as tile
from concourse import bacc

# Problem shapes (hardcoded per contract).
B, C_IN, C_OUT, H, W, KS, DLAT = 8, 512, 512, 32, 32, 3, 512
HP, WP = H + 2, W + 2            # zero-padded input plane: 34 x 34
NPIX = H * W                     # 1024 (unpadded, as transferred)
PCH_I = C_IN // 128              # 4 input-channel chunks of 128 partitions
PCH_O = C_OUT // 128             # 4 output-channel chunks
NTAPS = KS * KS                  # 9 kernel taps
NW = NTAPS * PCH_I               # 36 lhsT tiles of [128, C_OUT]
NHALF = 2                        # output pixels split into 2 PSUM banks of 512
HHALF = H // NHALF               # 16 output rows per half
N_CORES = 8

SQRT2 = 1.4142135623730951
LRELU_SLOPE = 0.2
EPS = 1e-8

# Matmul precision: "f32r" (fp32 storage, reduced-precision PE pass, 1 cyc/row),
# "bf16" (cast inputs, 1 cyc/row), "f32" (exact, 4 cyc/row).
MM_DTYPE = "f32r"

_NC_CACHE: dict = {}
_RUNNER_CACHE: dict = {}
_W_DEV_CACHE: dict = {}


def _mm_dts(mm_dtype):
    if mm_dtype == "f32r":
        return mybir.dt.float32r
    if mm_dtype == "bf16":
        return mybir.dt.bfloat16
    return mybir.dt.float32


def _emit_conv(ctx, tc, o_d, w_d, x_d, sdm_d, mm_dtype):
    nc = tc.nc
    f32 = mybir.dt.float32
    mm_dt = _mm_dts(mm_dtype)

    singles = ctx.enter_context(tc.tile_pool(name="singles", bufs=1))
    psum = ctx.enter_context(tc.tile_pool(name="psum", bufs=8, space="PSUM"))
    outp = ctx.enter_context(tc.tile_pool(name="outp", bufs=4))

    # Input planes: DMA unpadded; zero only the pad border (GpSimd — otherwise
    # idle), then modulate the interior: xs = s * x.
    x_sb = singles.tile([128, PCH_I, H, W], f32)
    sdm_sb = singles.tile([128, PCH_I + PCH_O], f32)
    xs_sb = singles.tile([128, PCH_I, HP, WP], mm_dt)
    xs_z = xs_sb.bitcast(f32) if mm_dtype == "f32r" else xs_sb
    for c in range(PCH_I):
        nc.gpsimd.memset(xs_z[:, c, 0, :], 0.0)
        nc.gpsimd.memset(xs_z[:, c, HP - 1, :], 0.0)
        nc.gpsimd.memset(xs_z[:, c, 1 : HP - 1, 0:1], 0.0)
        nc.gpsimd.memset(xs_z[:, c, 1 : HP - 1, WP - 1 : WP], 0.0)

    # Conv weights in lhsT layout, in-chunk-major: ti = kc*9 + tap.
    # First chunk arrives as 9 small per-tap DMAs (PE can start ~immediately);
    # remaining chunks as 3 large block DMAs (cheap to issue, arrive in time).
    w_sb = singles.tile([128, NW, C_OUT], mm_dt)

    def x_load(c):
        nc.sync.dma_start(x_sb[:, c, :, :], x_d[c].rearrange("p (h w) -> p h w", h=H))
        nc.vector.tensor_scalar_mul(
            xs_sb[:, c, 1 : H + 1, 1 : W + 1], x_sb[:, c, :, :], sdm_sb[:, c : c + 1]
        )

    # Interleave so the critical chain (x0 -> scale -> first matmul) issues
    # first; the tiny s/d vector DMA rides along second.
    nc.sync.dma_start(
        x_sb[:, 0, :, :], x_d[0].rearrange("p (h w) -> p h w", h=H)
    )
    nc.sync.dma_start(sdm_sb[:, :], sdm_d[:, :])
    nc.vector.tensor_scalar_mul(
        xs_sb[:, 0, 1 : H + 1, 1 : W + 1], x_sb[:, 0, :, :], sdm_sb[:, 0:1]
    )
    nc.sync.dma_start(w_sb[:, 0, :], w_d[0, 0])
    x_load(1)
    nc.sync.dma_start(w_sb[:, 1, :], w_d[0, 1])
    x_load(2)
    nc.sync.dma_start(w_sb[:, 2, :], w_d[0, 2])
    x_load(3)
    for t9 in range(3, NTAPS):
        nc.sync.dma_start(w_sb[:, t9, :], w_d[0, t9])
    for kc in range(1, PCH_I):
        nc.sync.dma_start(
            w_sb[:, kc * NTAPS : (kc + 1) * NTAPS, :],
            w_d[kc].rearrange("t p n -> p t n"),
        )

    # 8 accumulation groups: (out-chunk mc, pixel-half nh), one PSUM bank each.
    ps = [[psum.tile([128, HHALF * W], f32, name="ps", tag="ps") for _ in range(NHALF)]
          for _ in range(PCH_O)]

    def mm(mc, nh, kc, t9, start, stop):
        kh, kw = divmod(t9, KS)
        r0 = nh * HHALF + kh
        nc.tensor.matmul(
            ps[mc][nh][:, :],
            lhsT=w_sb[:, kc * NTAPS + t9, mc * 128 : (mc + 1) * 128],
            rhs=xs_sb[:, kc, r0 : r0 + HHALF, kw : kw + W],
            start=start,
            stop=stop,
        )

    # Phase A: chunks 0..2 in weight-arrival order, all 8 groups in parallel
    # (PE stays dense and consumes w tiles as the DMAs land).
    for kc in range(PCH_I - 1):
        for t9 in range(NTAPS):
            for mc in range(PCH_O):
                for nh in range(NHALF):
                    mm(mc, nh, kc, t9, start=(kc == 0 and t9 == 0), stop=False)

    # Phase B: last chunk group-by-group so groups finish staggered ~1.9us
    # apart and the demod+store drains overlap with remaining PE work.
    kc = PCH_I - 1
    for mc in range(PCH_O):
        for nh in range(NHALF):
            for t9 in range(NTAPS):
                mm(mc, nh, kc, t9, start=False, stop=(t9 == NTAPS - 1))
            ob = outp.tile([128, HHALF * W], f32, name="ob", tag="ob")
            nc.vector.tensor_scalar_mul(
                ob[:, :], ps[mc][nh][:, :], sdm_sb[:, PCH_I + mc : PCH_I + mc + 1]
            )
            nc.sync.dma_start(
                o_d[mc * 128 : (mc + 1) * 128, nh * HHALF * W : (nh + 1) * HHALF * W],
                ob[:, :],
            )


def _build(mm_dtype):
    f32 = mybir.dt.float32
    w_io_dt = _mm_dts(mm_dtype) if mm_dtype != "f32" else f32
    nc = bacc.Bacc("TRN2", target_bir_lowering=False, debug=False)
    w_d = nc.dram_tensor(
        "w", [PCH_I, NTAPS, 128, C_OUT], w_io_dt, kind="ExternalInput"
    ).ap()
    x_d = nc.dram_tensor("x", [PCH_I, 128, NPIX], f32, kind="ExternalInput").ap()
    sdm_d = nc.dram_tensor(
        "sdm", [128, PCH_I + PCH_O], f32, kind="ExternalInput"
    ).ap()
    o_d = nc.dram_tensor("o", [C_OUT, H * W], f32, kind="ExternalOutput").ap()

    with tile.TileContext(nc) as tc:
        with ExitStack() as ctx:
            _emit_conv(ctx, tc, o_d, w_d, x_d, sdm_d, mm_dtype)
    nc.compile()
    return nc


def get_nc(mm_dtype=MM_DTYPE):
    if mm_dtype not in _NC_CACHE:
        _NC_CACHE[mm_dtype] = _build(mm_dtype)
    return _NC_CACHE[mm_dtype]


def prepare_inputs(x, y, weight, mod_weight, bias, mm_dtype=MM_DTYPE):
    """Host-side prep: style/demod vectors + device data layouts.

    Returns a dict of *global* arrays: "w" replicated, others concatenated
    along axis 0 across the 8 cores (one sample per core).
    """
    x = np.ascontiguousarray(np.asarray(x, np.float32))
    y = np.asarray(y, np.float32)
    weight = np.asarray(weight, np.float32)
    mod_weight = np.asarray(mod_weight, np.float32)
    bias = np.asarray(bias, np.float32)

    # Style s[b,i] = sqrt(2) * leaky_relu(y @ (mod_weight * dlat^-0.5).T + bias).
    s = y @ (mod_weight.T * np.float32(DLAT ** -0.5))
    s = s + bias[None, :]
    s = np.where(s >= 0, s, LRELU_SLOPE * s).astype(np.float32) * np.float32(SQRT2)

    # Demod d[b,o] = rsqrt(sum_i s^2 * sum_t w^2 + eps) (exact refactoring).
    w64 = weight.astype(np.float64)
    w2 = (w64 * w64).sum(axis=(2, 3))                       # [C_OUT, C_IN]
    den = (s.astype(np.float64) ** 2) @ w2.T + EPS          # [B, C_OUT]
    dmod = (1.0 / np.sqrt(den)).astype(np.float32)

    # lhsT weights [kh,kw,i,o], in-chunk-major: [PCH_I, NTAPS, 128, C_OUT].
    wT = np.ascontiguousarray(
        weight.transpose(2, 3, 1, 0)
        .reshape(NTAPS, PCH_I, 128, C_OUT)
        .transpose(1, 0, 2, 3)
    )
    if mm_dtype == "bf16":
        import ml_dtypes

        wT = wT.astype(ml_dtypes.bfloat16)

    xg = x.reshape(B * PCH_I, 128, NPIX)                    # [32, 128, 1024]
    s_t = s.reshape(B, PCH_I, 128).transpose(0, 2, 1)       # [B, 128, 4]
    d_t = dmod.reshape(B, PCH_O, 128).transpose(0, 2, 1)    # [B, 128, 4]
    sdm = np.ascontiguousarray(np.concatenate([s_t, d_t], axis=2)).reshape(
        B * 128, PCH_I + PCH_O
    )

    return {"w": wT, "x": xg, "sdm": sdm}


def per_core_map(global_in, core):
    """Slice the global input dict into one core's input map (for CoreSim)."""
    return {
        "w": global_in["w"],
        "x": global_in["x"][core * PCH_I : (core + 1) * PCH_I],
        "sdm": global_in["sdm"][core * 128 : (core + 1) * 128],
    }


def _make_runner(nc):
    """Persistent jitted SPMD executor: weight replicated, rest batch-sharded,
    donated output zeros created on-device (nothing extra over the wire)."""
    import jax
    import jax.numpy as jnp
    from jax.sharding import Mesh, PartitionSpec
    from jax.experimental.shard_map import shard_map

    from concourse.bass2jax import (
        _bass_exec_p,
        install_neuronx_cc_hook,
        partition_id_tensor,
    )

    install_neuronx_cc_hook()

    partition_name = (
        nc.partition_id_tensor.name if nc.partition_id_tensor is not None else None
    )
    in_names: list = []
    out_names: list = []
    out_avals: list = []
    for alloc in nc.m.functions[0].allocations:
        if not isinstance(alloc, mybir.MemoryLocationSet):
            continue
        name = alloc.memorylocations[0].name
        if alloc.kind == "ExternalInput":
            if name != partition_name:
                in_names.append(name)
        elif alloc.kind == "ExternalOutput":
            out_names.append(name)
            out_avals.append(
                jax.core.ShapedArray(tuple(alloc.tensor_shape), mybir.dt.np(alloc.dtype))
            )
    all_in_names = list(in_names) + list(out_names)
    if partition_name is not None:
        all_in_names.append(partition_name)

    def _body(*args):
        operands = list(args)
        if partition_name is not None:
            operands.append(partition_id_tensor())
        outs = _bass_exec_p.bind(
            *operands,
            out_avals=tuple(out_avals),
            in_names=tuple(all_in_names),
            out_names=tuple(out_names),
            lowering_input_output_aliases=(),
            sim_require_finite=True,
            sim_require_nnan=True,
            nc=nc,
        )
        return tuple(outs)

    devices = jax.devices()[:N_CORES]
    assert len(devices) == N_CORES, f"need {N_CORES} devices, got {len(devices)}"
    mesh = Mesh(np.asarray(devices), ("core",))
    spec_by_name = {n: PartitionSpec("core") for n in in_names}
    spec_by_name["w"] = PartitionSpec()          # replicated: one copy over the wire
    in_specs = tuple(spec_by_name[n] for n in in_names) + (
        (PartitionSpec("core"),) * len(out_names)
    )
    out_specs = (PartitionSpec("core"),) * len(out_names)
    fn = jax.jit(
        shard_map(
            _body, mesh=mesh, in_specs=in_specs, out_specs=out_specs, check_rep=False
        )
    )

    # Output "seed" buffers: the bass_exec custom call requires one parameter
    # per ExternalOutput. Our kernel writes every output element, so they only
    # need to exist, not be re-zeroed per call — create once on-device.
    from jax.sharding import NamedSharding

    def _mk_zeros():
        return tuple(
            jnp.zeros((N_CORES * a.shape[0],) + a.shape[1:], a.dtype)
            for a in out_avals
        )

    zeros_sh = tuple(
        NamedSharding(mesh, PartitionSpec("core")) for _ in out_avals
    )
    out_seeds = jax.jit(_mk_zeros, out_shardings=zeros_sh)()
    return fn, in_names, out_names, mesh, out_seeds


def get_runner(mm_dtype=MM_DTYPE):
    if mm_dtype not in _RUNNER_CACHE:
        _RUNNER_CACHE[mm_dtype] = _make_runner(get_nc(mm_dtype))
    return _RUNNER_CACHE[mm_dtype]


def _w_device(wT, mesh):
    """Cache the replicated weight on-device across calls (keyed by content)."""
    import jax
    from jax.sharding import NamedSharding, PartitionSpec

    key = hashlib.blake2b(wT.tobytes(), digest_size=16).hexdigest()
    hit = _W_DEV_CACHE.get(key)
    if hit is None:
        sh = NamedSharding(mesh, PartitionSpec())
        _W_DEV_CACHE.clear()
        hit = _W_DEV_CACHE[key] = jax.device_put(wT, sh)
    return hit


def kernel(x, y, weight, mod_weight, bias):
    gin = prepare_inputs(x, y, weight, mod_weight, bias, MM_DTYPE)
    fn, in_names, out_names, mesh, out_seeds = get_runner(MM_DTYPE)
    gin["w"] = _w_device(gin["w"], mesh)
    outs = fn(*[gin[n] for n in in_names], *out_seeds)
    out = np.asarray(outs[out_names.index("o")])             # [8*512, 1024]
    return out.reshape(B, C_OUT, H, W).astype(np.float32, copy=False)
